# revision 33
# baseline (speedup 1.0000x reference)
"""Trainium2 Bass kernel for nn_Attention_78786880078278.

Dense causal multi-head attention layer (QKV proj + RoPE + causal softmax
attention + output proj), sharded over 8 NeuronCores:
  - NEFF 1 (head-parallel): each core computes QKV projections, RoPE and
    causal attention for its 2 heads (x 2 batches), producing per-head
    attention outputs.
  - host: pure relayout (gather + transpose + dtype cast) of tensors
    between the two device programs.
  - NEFF 2 (token-parallel): each core computes the output projection for
    its 512-token slice.

All matmuls run in bf16 with fp32 PSUM accumulation.  The bf16 casts of the
static operands (x, weights) are done host-side - numerically identical to
the on-device casts the previous version performed, but they halve the HBM
traffic and remove ~80us of DVE cast work per core.  Both NEFFs open with a
short burst of throwaway matmuls so the PE HAM clock-gate is already warm
(2.4 GHz) when the first real matmul issues.
"""

import contextlib
import ctypes
import hashlib
import json
import math
import os
import shutil
import sys
import types

import numpy as np

# ---------------------------------------------------------------------------
# environment fixups
# ---------------------------------------------------------------------------

for _p in ("/opt/trn_rl_repo",):
    if _p not in sys.path and os.path.isdir(_p):
        sys.path.append(_p)

import concourse.bass as bass  # noqa: E402
import concourse.bass2jax as bass2jax  # noqa: E402
import concourse.mybir as mybir  # noqa: E402
import concourse.tile as tile  # noqa: E402
from concourse.bass_utils import run_bass_kernel_spmd  # noqa: E402

F32 = mybir.dt.float32
BF16 = mybir.dt.bfloat16
NP_BF16 = mybir.dt.np(mybir.dt.bfloat16)

_NEFF_CACHE_DIR = os.environ.get("NEFF_CACHE_DIR", "/tmp/neff_cache")


def _install_compile_fixups():
    """(1) Split multi-wait instructions: this walrus build encodes a single
    sync-wait slot per instruction and rejects Tile's final multi-wait drain.
    (2) Cache compiled NEFFs by BIR hash so repeated runs skip walrus."""
    if getattr(bass2jax, "_attn_fixup_installed", False):
        return
    orig = bass2jax.compile_bir_kernel

    def _fix_multiwait(bir_bytes):
        bir = json.loads(bir_bytes)
        changed = False
        for fn in bir.get("functions", []):
            for blk in fn.get("basic_blocks", fn.get("blocks", [])):
                new_insts = []
                for inst in blk.get("instructions", []):
                    si = inst.get("sync_info") or {}
                    waits = si.get("on_wait") or []
                    if len(waits) > 1:
                        changed = True
                        for i, w in enumerate(waits[:-1]):
                            pre = {
                                "name": f"{inst['name']}_w{i}",
                                "opcode": "Drain",
                                "engine": inst["engine"],
                                "ins": [],
                                "outs": [],
                                "sync_info": {"on_wait": [w], "on_update": []},
                            }
                            if "debug" in inst:
                                pre["debug"] = inst["debug"]
                            if "is_reset_sema" in inst:
                                pre["is_reset_sema"] = False
                            new_insts.append(pre)
                        si["on_wait"] = [waits[-1]]
                        inst["sync_info"] = si
                    new_insts.append(inst)
                blk["instructions"] = new_insts
        return json.dumps(bir).encode() if changed else bir_bytes

    def _patched(bir_json, tmpdir, neff_name="file.neff"):
        fixed = _fix_multiwait(bir_json)
        key = hashlib.sha256(fixed).hexdigest()[:24]
        cached = os.path.join(_NEFF_CACHE_DIR, f"{key}.neff")
        target = os.path.join(tmpdir, neff_name)
        if os.path.exists(cached):
            shutil.copy(cached, target)
            return target
        path = orig(fixed, tmpdir, neff_name)
        try:
            os.makedirs(_NEFF_CACHE_DIR, exist_ok=True)
            shutil.copy(path, cached)
        except OSError:
            pass
        return path

    bass2jax.compile_bir_kernel = _patched
    bass2jax._attn_fixup_installed = True


def _install_ntff_hook():
    """Register the NTFF profiling hook (used only when BASS_TRACE=1)."""
    try:
        import antenv
    except ImportError:
        return
    if "antenv.axon_hooks" in sys.modules:
        return
    so_path = "/opt/axon/libaxon_pjrt.so"
    try:
        lib = ctypes.CDLL(so_path)
    except OSError:
        return
    if not hasattr(lib, "axon_start_nrt_profile"):
        return
    lib.axon_start_nrt_profile.argtypes = [
        ctypes.POINTER(ctypes.c_int64),
        ctypes.c_size_t,
    ]
    lib.axon_start_nrt_profile.restype = ctypes.c_int64
    lib.axon_stop_nrt_profile.argtypes = [ctypes.c_char_p]
    lib.axon_stop_nrt_profile.restype = ctypes.c_int64

    @contextlib.contextmanager
    def _hook(output_dir, device_ids):
        import jax

        jax.devices()
        if device_ids:
            ids = (ctypes.c_int64 * len(device_ids))(*device_ids)
            rc = lib.axon_start_nrt_profile(ids, len(device_ids))
        else:
            rc = lib.axon_start_nrt_profile(None, 0)
        if rc != 0:
            raise RuntimeError(f"axon_start_nrt_profile rc={rc}")
        try:
            yield
        finally:
            n = lib.axon_stop_nrt_profile(str(output_dir).encode())
            print(f"profile: {n} file(s) in {output_dir}", file=sys.stderr)

    mod = types.ModuleType("antenv.axon_hooks")
    mod.get_axon_ntff_profile_hook = lambda: _hook
    mod.set_axon_ntff_profile_hook = lambda h: None
    sys.modules["antenv.axon_hooks"] = mod
    antenv.axon_hooks = mod


_install_compile_fixups()
_install_ntff_hook()

# ---------------------------------------------------------------------------
# problem constants (hardcoded per the task spec)
# ---------------------------------------------------------------------------

HIDDEN = 2048
HEADS = 16
HD = 128  # head dim
B = 2
S = 2048
N_CORES = 8
HPC = HEADS // N_CORES  # heads per core = 2
SPAN = 512
NSPANS = S // SPAN  # 4 query spans per batch
KT = HIDDEN // 128  # 16 contraction tiles
TT = S // 128  # 16 token tiles per batch
SCALE = 1.0 / math.sqrt(HD)
TOK_SLICE = (B * S) // N_CORES  # 512 tokens per core in NEFF 2

LAST_RESULTS = []  # BassKernelResults of the most recent kernel() call


# ---------------------------------------------------------------------------
# NEFF 1: QKV projections + RoPE + causal attention for 2 heads x 2 batches
# ---------------------------------------------------------------------------

def build_attn_nc():
    nc = bass.Bass(target_bir_lowering=False, debug=False)

    # all inputs partition-major, pre-cast on host
    xP = nc.dram_tensor("xP", [B, 128, KT, S], BF16, kind="ExternalInput")
    wqP = nc.dram_tensor("wqP", [128, KT, HPC * HD], BF16, kind="ExternalInput")
    wkP = nc.dram_tensor("wkP", [128, KT, HPC * HD], BF16, kind="ExternalInput")
    wvP = nc.dram_tensor("wvP", [128, KT, HPC * HD], BF16, kind="ExternalInput")
    cosT = nc.dram_tensor("cosT", [HD, S], BF16, kind="ExternalInput")
    sinT = nc.dram_tensor("sinT", [HD, S], BF16, kind="ExternalInput")  # sign-folded
    maskd = nc.dram_tensor("mask", [128, 128], BF16, kind="ExternalInput")
    attnout = nc.dram_tensor(
        "attnout", [B, NSPANS, 128, HPC, 4, 128], BF16, kind="ExternalOutput"
    )

    with tile.TileContext(nc) as tc:
        with (
            tc.tile_pool(name="warm", bufs=1) as warm,
            tc.tile_pool(name="persist", bufs=1) as persist,
            tc.tile_pool(name="xpool", bufs=3) as xpool,
            tc.tile_pool(name="rope", bufs=2) as rope,
            tc.tile_pool(name="epool", bufs=32) as epool,
            tc.tile_pool(name="opool", bufs=2) as opool,
            tc.tile_pool(name="rpool", bufs=4) as rpool,
            tc.tile_pool(name="ps_qk", bufs=4, space="PSUM") as ps_qk,
            tc.tile_pool(name="ps_sc", bufs=2, space="PSUM") as ps_sc,
        ):
            # ---------------- persistent tiles ----------------
            wq_bf = persist.tile([128, KT, HPC * HD], BF16, tag="wq_bf")
            wk_bf = persist.tile([128, KT, HPC * HD], BF16, tag="wk_bf")
            wv_bf = persist.tile([128, KT, HPC * HD], BF16, tag="wv_bf")
            cos_sb = persist.tile([HD, S], BF16, tag="cos_sb")
            sin_sb = persist.tile([HD, S], BF16, tag="sin_sb")
            mask_bf = persist.tile([128, 128], BF16, tag="mask_bf")
            # q and k share one tile (dim1: 0=q, 1=k) so RoPE's rotate-half
            # swap needs one DMA pair per (span, head) instead of two
            qk_sb = persist.tile([HD, 2, B, HPC, S], BF16, tag="qk_sb")
            # v with an appended ones column (denominator trick)
            v_sb = persist.tile([128, B, TT, HPC, HD + 1], BF16, tag="v_sb")

            # -------- PE warm-up: garbage matmuls while DMA ramps --------
            # HAM un-throttles (1.2 -> 2.4 GHz) only after ~3.4us of
            # sustained PE activity; burn that in during the initial DMA.
            # matmuls on uninitialized SBUF garbage: results land in a PSUM
            # bank that is never read (the first real accumulation into the
            # reused bank has start=True, which overwrites).  No producer
            # dependency, so these issue the moment the engine comes up.
            wtile = warm.tile([128, 644], BF16, tag="wtile")
            # one-column write allocates the tile; the matmuls read a
            # disjoint (uninitialized) region so they have no producer dep
            nc.vector.memset(wtile[:, 0:1], 0.0)
            ps_w = ps_qk.tile([128, SPAN], F32, tag="qk")
            for i in range(24):
                nc.tensor.matmul(
                    ps_w[:], wtile[:, 4:132], wtile[:, 132:644],
                    start=True, stop=True,
                )

            def load_weight(wdram, wbf, pieces=4):
                kstep = KT // pieces
                for p in range(pieces):
                    nc.sync.dma_start(
                        wbf[:, p * kstep:(p + 1) * kstep, :],
                        wdram[:, p * kstep:(p + 1) * kstep, :],
                    )

            def load_xspan(b, span, pieces=1):
                xspan = xpool.tile([128, KT, SPAN], BF16, tag="x_bf")
                kstep = KT // pieces
                sl = slice(span * SPAN, (span + 1) * SPAN)
                for p in range(pieces):
                    ks = slice(p * kstep, (p + 1) * kstep)
                    nc.sync.dma_start(xspan[:, ks, :], xP[b, :, ks, sl])
                return xspan

            def qk_span(b, span, xspan):
                sl = slice(span * SPAN, (span + 1) * SPAN)
                for h in range(HPC):
                    hsl = slice(h * HD, (h + 1) * HD)
                    pf2 = rope.tile([128, 2, SPAN], BF16, tag="pf")
                    for qk, wbf in ((0, wq_bf), (1, wk_bf)):
                        ps = ps_qk.tile([128, SPAN], F32, tag="qk")
                        for kt in range(KT):
                            nc.tensor.matmul(
                                ps[:],
                                wbf[:, kt, hsl],
                                xspan[:, kt, :],
                                start=(kt == 0),
                                stop=(kt == KT - 1),
                            )
                        nc.scalar.copy(pf2[:, qk, :], ps[:])
                    # RoPE on q and k together: out = p*cos + rot(p)*sin
                    # (bf16 math: ~0.2% extra error, 2x DVE throughput)
                    rot2 = rope.tile([128, 2, SPAN], BF16, tag="rot")
                    nc.sync.dma_start(rot2[0:64], pf2[64:128])
                    nc.sync.dma_start(rot2[64:128], pf2[0:64])
                    cos_b = cos_sb[:, sl].unsqueeze(1).broadcast_to(
                        [HD, 2, SPAN])
                    sin_b = sin_sb[:, sl].unsqueeze(1).broadcast_to(
                        [HD, 2, SPAN])
                    nc.vector.tensor_mul(pf2[:], pf2[:], cos_b)
                    nc.vector.tensor_mul(rot2[:], rot2[:], sin_b)
                    nc.vector.tensor_add(
                        qk_sb[:, :, b, h, sl], pf2[:], rot2[:])

            def v_span(b, span, xspan):
                # V projection (natural [token, head*hd] layout).  Emitted
                # AFTER the span's score matmuls: those only need Q/K, and
                # the V matmuls give ScalarE time to chew through the exps.
                for j in range(4):
                    tt = span * 4 + j
                    psv = ps_qk.tile([128, HPC * HD], F32, tag="qk")
                    for kt in range(KT):
                        nc.tensor.matmul(
                            psv[:],
                            xspan[:, kt, j * 128:(j + 1) * 128],
                            wv_bf[:, kt, :],
                            start=(kt == 0),
                            stop=(kt == KT - 1),
                        )
                    nc.vector.tensor_copy(
                        v_sb[:, b, tt, :, 0:HD],
                        psv[:].rearrange("p (h d) -> p h d", h=HPC),
                    )

            def attn_scores(b, h, s):
                q0 = s * SPAN
                nkt = 4 * s + 4  # causal: k tiles 0 .. 4s+3 (always even)
                es = []
                for kp in range(nkt // 2):
                    # two k-tiles share a 2-bank PSUM: one ACTIVATE covers both
                    # exps, amortizing ScalarE's per-op overhead.  Diagonal
                    # k-tiles (jd>0) only compute the causal q-suffix; the
                    # skipped region of the PSUM/e2 tile is never read.
                    psc = ps_sc.tile([128, 2 * SPAN], F32, tag="sc")
                    for half in range(2):
                        kt = 2 * kp + half
                        c0 = max(0, (kt - 4 * s)) * 128  # causal q-col start
                        nc.tensor.matmul(
                            psc[:, half * SPAN + c0:(half + 1) * SPAN],
                            qk_sb[:, 1, b, h, kt * 128:(kt + 1) * 128],
                            qk_sb[:, 0, b, h, q0 + c0:q0 + SPAN],
                            start=True,
                            stop=True,
                        )
                    e2 = epool.tile([128, 2 * SPAN], BF16, tag="e")
                    ec0 = max(0, (2 * kp - 4 * s)) * 128
                    nc.scalar.activation(
                        e2[:, ec0:], psc[:, ec0:],
                        mybir.ActivationFunctionType.Exp, scale=SCALE
                    )
                    for half in range(2):
                        kt = 2 * kp + half
                        jd = kt - 4 * s
                        base = half * SPAN
                        if jd >= 0:  # diagonal block: zero out k > q
                            nc.vector.tensor_mul(
                                e2[:, base + jd * 128:base + (jd + 1) * 128],
                                e2[:, base + jd * 128:base + (jd + 1) * 128],
                                mask_bf[:],
                            )
                        es.append(e2[:, base:base + SPAN])
                return es

            def attn_vmm(b, h, s, es, o_sb):
                for j in range(4):
                    last_kt = 4 * s + j
                    pso = ps_qk.tile([128, SPAN], F32, tag="qk")
                    for kt in range(last_kt + 1):
                        nc.tensor.matmul(
                            pso[:, 0:HD + 1],
                            es[kt][:, j * 128:(j + 1) * 128],
                            v_sb[:, b, kt, h, :],
                            start=(kt == 0),
                            stop=(kt == last_kt),
                        )
                    recip = rpool.tile([128, 1], F32, tag="recip")
                    nc.vector.reciprocal(recip[:], pso[:, HD:HD + 1])
                    nc.vector.tensor_scalar_mul(
                        o_sb[:, h, j, :], pso[:, 0:HD], recip[:]
                    )

            # ---------------- emission schedule ----------------
            # Spans are processed in global order (batch 0 then batch 1);
            # each span's attention chunks run right after its QKV so early
            # x-span DMA waits are filled with attention matmuls instead of
            # idling the PE (which would also re-throttle the HAM clock
            # gate).  Scores stay one chunk ahead of attn@V so the ScalarE
            # exps hide behind TensorE work.
            spans = [(b, s) for b in range(B) for s in range(NSPANS)]
            x_first = xpool.tile([128, KT, SPAN], BF16, tag="x_bf")
            xtiles = {spans[0]: x_first}
            # each dma_start costs ~0.65us of serialized posting on the Sync
            # queue, so early loads are few and coarse: the posting rate, not
            # HBM bandwidth, set the old warm-up critical path
            nc.sync.dma_start(wq_bf[:, 0:8, :], wqP[:, 0:8, :])
            nc.sync.dma_start(x_first[:], xP[0, :, :, 0:SPAN])
            nc.sync.dma_start(wq_bf[:, 8:16, :], wqP[:, 8:16, :])
            nc.sync.dma_start(wk_bf[:, 0:8, :], wkP[:, 0:8, :])
            nc.sync.dma_start(wk_bf[:, 8:16, :], wkP[:, 8:16, :])
            nc.sync.dma_start(cos_sb[:], cosT[:])
            nc.sync.dma_start(sin_sb[:], sinT[:])
            nc.sync.dma_start(mask_bf[:], maskd[:])
            load_weight(wvP, wv_bf, pieces=1)
            xtiles[spans[1]] = load_xspan(*spans[1])
            nc.vector.memset(v_sb[:, :, :, :, HD], 1.0)

            chunks = [(b, h, s) for (b, s) in spans for h in range(HPC)]
            es_map = {}
            o_tiles = {}
            scored = 0  # chunks whose scores have been emitted
            done = 0    # chunks whose attn@V has been emitted

            def emit_vmm(c):
                cb, ch, cs = chunks[c]
                key = (cb, cs)
                if key not in o_tiles:
                    o_tiles[key] = opool.tile([128, HPC, 4, 128], BF16,
                                              tag="o", name=f"o_{cb}_{cs}")
                attn_vmm(cb, ch, cs, es_map.pop(c), o_tiles[key])
                if ch == HPC - 1:  # both chunks of the span done: one DMA
                    nc.sync.dma_start(attnout[cb, cs], o_tiles.pop(key)[:])
            for sp, (b, s) in enumerate(spans):
                xspan = xtiles.pop((b, s))
                qk_span(b, s, xspan)
                if sp < len(spans) - 1:
                    v_span(b, s, xspan)
                    if sp + 2 < len(spans):
                        xtiles[spans[sp + 2]] = load_xspan(*spans[sp + 2])
                    for _ in range(HPC):
                        es_map[scored] = attn_scores(*chunks[scored])
                        scored += 1
                        if scored - done >= 4:
                            emit_vmm(done)
                            done += 1
                else:
                    # last span: score its chunks BEFORE the V projection so
                    # the final (ScalarE-bound) exp stream hides behind the V
                    # matmuls instead of stalling the attn@V drain
                    for _ in range(HPC):
                        es_map[scored] = attn_scores(*chunks[scored])
                        scored += 1
                        if scored - done >= 3:
                            emit_vmm(done)
                            done += 1
                    v_span(b, s, xspan)
            while done < len(chunks):
                emit_vmm(done)
                done += 1
    return nc


# ---------------------------------------------------------------------------
# NEFF 2: output projection, token-parallel
# ---------------------------------------------------------------------------

def build_oproj_nc():
    """out[tok, hout] = attnT.T @ WoT on a 2x4 (token-half x hout-quarter)
    core grid: per-core inputs are 8 MiB of bf16 attention state and 2 MiB of
    bf16 Wo columns, streamed K-tile by K-tile so the loads hide under
    matmuls."""
    nc = bass.Bass(target_bir_lowering=False, debug=False)

    TOKS = (B * S) // 2   # 2048 tokens per core (token half)
    HOUT = HIDDEN // 4    # 512 output channels per core (hout quarter)
    aP = nc.dram_tensor("aP", [128, KT, TOKS], BF16, kind="ExternalInput")
    woP = nc.dram_tensor("woP", [128, KT, HOUT], BF16, kind="ExternalInput")
    out = nc.dram_tensor("out", [TOKS, HOUT], F32, kind="ExternalOutput")

    with tile.TileContext(nc) as tc:
        with (
            tc.tile_pool(name="warm", bufs=1) as warm,
            tc.tile_pool(name="persist", bufs=1) as persist,
            tc.tile_pool(name="outp", bufs=3) as outp,
            tc.tile_pool(name="psum", bufs=8, space="PSUM") as psum,
        ):
            a_bf = persist.tile([128, KT, TOKS], BF16, tag="a_bf")
            wo_bf = persist.tile([128, KT, HOUT], BF16, tag="wo_bf")

            # PE warm-up while input DMA ramps
            wtile = warm.tile([128, 644], BF16, tag="wtile")
            nc.vector.memset(wtile[:, 0:1], 0.0)
            ps_w = psum.tile([128, HOUT], F32, tag="ps", name="ps_warm")
            for i in range(16):
                nc.tensor.matmul(
                    ps_w[:], wtile[:, 4:132], wtile[:, 132:644],
                    start=True, stop=True,
                )

            def flush(m_tiles, fuse=False):
                ms = list(m_tiles)
                if fuse:  # one copy+DMA pair for the whole group (tail trim)
                    o2 = outp.tile([128, len(ms), HOUT], F32, tag="o2",
                                   name=f"o2_{ms[0]}")
                    for k, m in enumerate(ms):
                        nc.vector.tensor_copy(o2[:, k, :], ps_grid[m][:])
                    dst = out[ms[0] * 128:(ms[0] + len(ms)) * 128, :]
                    nc.sync.dma_start(
                        dst.rearrange("(a p) h -> p a h", p=128), o2[:]
                    )
                    return
                for m in ms:
                    o = outp.tile([128, HOUT], F32, tag="o", name=f"o_{m}")
                    nc.vector.tensor_copy(o[:], ps_grid[m][:])
                    nc.sync.dma_start(out[m * 128:(m + 1) * 128, :], o[:])

            # pass 1: token tiles 0-7, streaming Wo in per K tile (JIT)
            ps_grid = {m: psum.tile([128, HOUT], F32, tag="ps", name=f"ps_{m}")
                       for m in range(8)}
            nc.sync.dma_start(a_bf[:, 0, :], aP[:, 0, :])
            nc.sync.dma_start(wo_bf[:, 0, :], woP[:, 0, :])
            nc.sync.dma_start(a_bf[:, 1, :], aP[:, 1, :])
            for kt in range(KT):
                if kt + 1 < KT:
                    nc.sync.dma_start(wo_bf[:, kt + 1, :], woP[:, kt + 1, :])
                if kt + 2 < KT:
                    nc.sync.dma_start(a_bf[:, kt + 2, :], aP[:, kt + 2, :])
                for m in range(8):
                    nc.tensor.matmul(
                        ps_grid[m][:],
                        a_bf[:, kt, m * 128:(m + 1) * 128],
                        wo_bf[:, kt, :],
                        start=(kt == 0),
                        stop=(kt == KT - 1),
                    )
            flush(range(8))
            # passes 2a/2b: token tiles 8-11 then 12-15 from the cached bf16
            # weights; splitting lets the first flush DMA overlap the second
            # sub-pass instead of sitting in the kernel tail
            for lo, n in ((8, 4), (12, 2), (14, 2)):
                ps_grid = {m: psum.tile([128, HOUT], F32, tag="ps",
                                        name=f"ps_{m}")
                           for m in range(lo, lo + n)}
                for kt in range(KT):
                    for m in range(lo, lo + n):
                        nc.tensor.matmul(
                            ps_grid[m][:],
                            a_bf[:, kt, m * 128:(m + 1) * 128],
                            wo_bf[:, kt, :],
                            start=(kt == 0),
                            stop=(kt == KT - 1),
                        )
                flush(range(lo, lo + n), fuse=(lo == 14))
    return nc


# ---------------------------------------------------------------------------
# host driver
# ---------------------------------------------------------------------------

_NC_CACHE = {}


def _get_ncs():
    if "attn" not in _NC_CACHE:
        _NC_CACHE["attn"] = build_attn_nc()
        _NC_CACHE["oproj"] = build_oproj_nc()
    return _NC_CACHE["attn"], _NC_CACHE["oproj"]


def _rope_tables():
    inv_freq = 1.0 / (10000.0 ** (np.arange(0, HD, 2, dtype=np.float32) / HD))
    t = np.arange(S, dtype=np.float32)
    freqs = np.einsum("i,j->ij", t, inv_freq)  # [S, HD/2]
    emb = np.concatenate([freqs, freqs], axis=-1)  # [S, HD]
    cos = np.cos(emb).astype(np.float32)
    sin = np.sin(emb).astype(np.float32)
    cosT = np.ascontiguousarray(cos.T)  # [HD, S]
    sinT = np.ascontiguousarray(sin.T)
    sinT_signed = sinT.copy()
    sinT_signed[0:64, :] *= -1.0  # fold rotate_half's negation into the table
    return cosT.astype(NP_BF16), sinT_signed.astype(NP_BF16)


def _w_partition_major(Wslice):
    """[256 outs, HIDDEN] f32 -> [128, KT, 256] bf16, partition-major."""
    # w[h, ko*128 + p] -> out[p, ko, h]
    return np.ascontiguousarray(
        Wslice.T.reshape(KT, 128, HPC * HD).transpose(1, 0, 2)
    ).astype(NP_BF16)


def kernel(x, Wq, Wk, Wv, Wo):
    x = np.asarray(x, dtype=np.float32)
    Wq = np.asarray(Wq, dtype=np.float32)
    Wk = np.asarray(Wk, dtype=np.float32)
    Wv = np.asarray(Wv, dtype=np.float32)
    Wo = np.asarray(Wo, dtype=np.float32)

    nc1, nc2 = _get_ncs()
    core_ids = list(range(N_CORES))
    trace = bool(os.environ.get("BASS_TRACE"))

    cosT, sinT_signed = _rope_tables()
    mask = np.triu(np.ones((128, 128), dtype=np.float32)).astype(
        NP_BF16
    )  # mask[k,q]=1 iff k<=q
    # x[b, t, ko*128+p] -> xP[b, p, ko, t]  (partition-major, bf16)
    xP = np.ascontiguousarray(
        x.reshape(B, S, KT, 128).transpose(0, 3, 2, 1)
    ).astype(NP_BF16)

    in_maps1 = []
    for c in range(N_CORES):
        csl = slice(c * HPC * HD, (c + 1) * HPC * HD)
        in_maps1.append(
            {
                "xP": xP,
                "wqP": _w_partition_major(Wq[csl, :]),
                "wkP": _w_partition_major(Wk[csl, :]),
                "wvP": _w_partition_major(Wv[csl, :]),
                "cosT": cosT,
                "sinT": sinT_signed,
                "mask": mask,
            }
        )

    LAST_RESULTS.clear()
    res1 = run_bass_kernel_spmd(nc1, in_maps1, core_ids=core_ids, trace=trace)
    LAST_RESULTS.append(res1)

    # host relayout: per-head attention outputs -> attnT [HIDDEN, B*S]
    arr = np.stack([res1.results[c]["attnout"] for c in range(N_CORES)])
    # axes: (core, b, s, ql, h, qt, dl) -> d = core*256 + h*128 + dl,
    #       tok = b*2048 + s*512 + qt*128 + ql
    attnT = np.ascontiguousarray(
        arr.transpose(0, 4, 6, 1, 2, 5, 3).reshape(HIDDEN, B * S)
    )
    # attnT[ko*128+p, tok] -> aP[p, ko, tok] per token half
    aP_full = np.ascontiguousarray(
        attnT.reshape(KT, 128, B * S).transpose(1, 0, 2)
    )
    # Wo[ho, ko*128+p] -> woP[p, ko, ho] per hout quarter
    woP_full = np.ascontiguousarray(
        Wo.T.reshape(KT, 128, HIDDEN).transpose(1, 0, 2)
    ).astype(NP_BF16)

    TOKS = (B * S) // 2
    HOUT = HIDDEN // 4
    in_maps2 = []
    for c in range(N_CORES):
        ti, hj = c // 4, c % 4
        in_maps2.append(
            {
                "aP": np.ascontiguousarray(
                    aP_full[:, :, ti * TOKS:(ti + 1) * TOKS]
                ),
                "woP": np.ascontiguousarray(
                    woP_full[:, :, hj * HOUT:(hj + 1) * HOUT]
                ),
            }
        )
    res2 = run_bass_kernel_spmd(nc2, in_maps2, core_ids=core_ids, trace=trace)
    LAST_RESULTS.append(res2)

    out = np.empty((B * S, HIDDEN), dtype=np.float32)
    for c in range(N_CORES):
        ti, hj = c // 4, c % 4
        out[ti * TOKS:(ti + 1) * TOKS, hj * HOUT:(hj + 1) * HOUT] = (
            res2.results[c]["out"]
        )
    return np.ascontiguousarray(out.reshape(B, S, HIDDEN), dtype=np.float32)


# revision 34
# speedup vs baseline: 1.0084x; 1.0084x over previous
"""Trainium2 Bass kernel for nn_Attention_78786880078278.

Dense causal multi-head attention layer (QKV proj + RoPE + causal softmax
attention + output proj), sharded over 8 NeuronCores:
  - NEFF 1 (head-parallel): each core computes QKV projections, RoPE and
    causal attention for its 2 heads (x 2 batches), producing per-head
    attention outputs.
  - host: pure relayout (gather + transpose + dtype cast) of tensors
    between the two device programs.
  - NEFF 2 (token-parallel): each core computes the output projection for
    its 512-token slice.

All matmuls run in bf16 with fp32 PSUM accumulation.  The bf16 casts of the
static operands (x, weights) are done host-side - numerically identical to
the on-device casts the previous version performed, but they halve the HBM
traffic and remove ~80us of DVE cast work per core.  Both NEFFs open with a
short burst of throwaway matmuls so the PE HAM clock-gate is already warm
(2.4 GHz) when the first real matmul issues.
"""

import contextlib
import ctypes
import hashlib
import json
import math
import os
import shutil
import sys
import types

import numpy as np

# ---------------------------------------------------------------------------
# environment fixups
# ---------------------------------------------------------------------------

for _p in ("/opt/trn_rl_repo",):
    if _p not in sys.path and os.path.isdir(_p):
        sys.path.append(_p)

import concourse.bass as bass  # noqa: E402
import concourse.bass2jax as bass2jax  # noqa: E402
import concourse.mybir as mybir  # noqa: E402
import concourse.tile as tile  # noqa: E402
from concourse.bass_utils import run_bass_kernel_spmd  # noqa: E402

F32 = mybir.dt.float32
BF16 = mybir.dt.bfloat16
NP_BF16 = mybir.dt.np(mybir.dt.bfloat16)

_NEFF_CACHE_DIR = os.environ.get("NEFF_CACHE_DIR", "/tmp/neff_cache")


def _install_compile_fixups():
    """(1) Split multi-wait instructions: this walrus build encodes a single
    sync-wait slot per instruction and rejects Tile's final multi-wait drain.
    (2) Cache compiled NEFFs by BIR hash so repeated runs skip walrus."""
    if getattr(bass2jax, "_attn_fixup_installed", False):
        return
    orig = bass2jax.compile_bir_kernel

    def _fix_multiwait(bir_bytes):
        bir = json.loads(bir_bytes)
        changed = False
        for fn in bir.get("functions", []):
            for blk in fn.get("basic_blocks", fn.get("blocks", [])):
                new_insts = []
                for inst in blk.get("instructions", []):
                    si = inst.get("sync_info") or {}
                    waits = si.get("on_wait") or []
                    if len(waits) > 1:
                        changed = True
                        for i, w in enumerate(waits[:-1]):
                            pre = {
                                "name": f"{inst['name']}_w{i}",
                                "opcode": "Drain",
                                "engine": inst["engine"],
                                "ins": [],
                                "outs": [],
                                "sync_info": {"on_wait": [w], "on_update": []},
                            }
                            if "debug" in inst:
                                pre["debug"] = inst["debug"]
                            if "is_reset_sema" in inst:
                                pre["is_reset_sema"] = False
                            new_insts.append(pre)
                        si["on_wait"] = [waits[-1]]
                        inst["sync_info"] = si
                    new_insts.append(inst)
                blk["instructions"] = new_insts
        return json.dumps(bir).encode() if changed else bir_bytes

    def _patched(bir_json, tmpdir, neff_name="file.neff"):
        fixed = _fix_multiwait(bir_json)
        key = hashlib.sha256(fixed).hexdigest()[:24]
        cached = os.path.join(_NEFF_CACHE_DIR, f"{key}.neff")
        target = os.path.join(tmpdir, neff_name)
        if os.path.exists(cached):
            shutil.copy(cached, target)
            return target
        path = orig(fixed, tmpdir, neff_name)
        try:
            os.makedirs(_NEFF_CACHE_DIR, exist_ok=True)
            shutil.copy(path, cached)
        except OSError:
            pass
        return path

    bass2jax.compile_bir_kernel = _patched
    bass2jax._attn_fixup_installed = True


def _install_ntff_hook():
    """Register the NTFF profiling hook (used only when BASS_TRACE=1)."""
    try:
        import antenv
    except ImportError:
        return
    if "antenv.axon_hooks" in sys.modules:
        return
    so_path = "/opt/axon/libaxon_pjrt.so"
    try:
        lib = ctypes.CDLL(so_path)
    except OSError:
        return
    if not hasattr(lib, "axon_start_nrt_profile"):
        return
    lib.axon_start_nrt_profile.argtypes = [
        ctypes.POINTER(ctypes.c_int64),
        ctypes.c_size_t,
    ]
    lib.axon_start_nrt_profile.restype = ctypes.c_int64
    lib.axon_stop_nrt_profile.argtypes = [ctypes.c_char_p]
    lib.axon_stop_nrt_profile.restype = ctypes.c_int64

    @contextlib.contextmanager
    def _hook(output_dir, device_ids):
        import jax

        jax.devices()
        if device_ids:
            ids = (ctypes.c_int64 * len(device_ids))(*device_ids)
            rc = lib.axon_start_nrt_profile(ids, len(device_ids))
        else:
            rc = lib.axon_start_nrt_profile(None, 0)
        if rc != 0:
            raise RuntimeError(f"axon_start_nrt_profile rc={rc}")
        try:
            yield
        finally:
            n = lib.axon_stop_nrt_profile(str(output_dir).encode())
            print(f"profile: {n} file(s) in {output_dir}", file=sys.stderr)

    mod = types.ModuleType("antenv.axon_hooks")
    mod.get_axon_ntff_profile_hook = lambda: _hook
    mod.set_axon_ntff_profile_hook = lambda h: None
    sys.modules["antenv.axon_hooks"] = mod
    antenv.axon_hooks = mod


_install_compile_fixups()
_install_ntff_hook()

# ---------------------------------------------------------------------------
# problem constants (hardcoded per the task spec)
# ---------------------------------------------------------------------------

HIDDEN = 2048
HEADS = 16
HD = 128  # head dim
B = 2
S = 2048
N_CORES = 8
HPC = HEADS // N_CORES  # heads per core = 2
SPAN = 512
NSPANS = S // SPAN  # 4 query spans per batch
KT = HIDDEN // 128  # 16 contraction tiles
TT = S // 128  # 16 token tiles per batch
SCALE = 1.0 / math.sqrt(HD)
TOK_SLICE = (B * S) // N_CORES  # 512 tokens per core in NEFF 2

LAST_RESULTS = []  # BassKernelResults of the most recent kernel() call


# ---------------------------------------------------------------------------
# NEFF 1: QKV projections + RoPE + causal attention for 2 heads x 2 batches
# ---------------------------------------------------------------------------

def build_attn_nc():
    nc = bass.Bass(target_bir_lowering=False, debug=False)

    # all inputs partition-major, pre-cast on host
    xP = nc.dram_tensor("xP", [B, 128, KT, S], BF16, kind="ExternalInput")
    wqP = nc.dram_tensor("wqP", [128, KT, HPC * HD], BF16, kind="ExternalInput")
    wkP = nc.dram_tensor("wkP", [128, KT, HPC * HD], BF16, kind="ExternalInput")
    wvP = nc.dram_tensor("wvP", [128, KT, HPC * HD], BF16, kind="ExternalInput")
    cosT = nc.dram_tensor("cosT", [HD, S], BF16, kind="ExternalInput")
    sinT = nc.dram_tensor("sinT", [HD, S], BF16, kind="ExternalInput")  # sign-folded
    maskd = nc.dram_tensor("mask", [128, 128], BF16, kind="ExternalInput")
    attnout = nc.dram_tensor(
        "attnout", [B, NSPANS, 128, HPC, 4, 128], BF16, kind="ExternalOutput"
    )

    with tile.TileContext(nc) as tc:
        with (
            tc.tile_pool(name="warm", bufs=1) as warm,
            tc.tile_pool(name="persist", bufs=1) as persist,
            tc.tile_pool(name="xpool", bufs=3) as xpool,
            tc.tile_pool(name="rope", bufs=2) as rope,
            tc.tile_pool(name="epool", bufs=32) as epool,
            tc.tile_pool(name="opool", bufs=2) as opool,
            tc.tile_pool(name="rpool", bufs=4) as rpool,
            tc.tile_pool(name="ps_qk", bufs=4, space="PSUM") as ps_qk,
            tc.tile_pool(name="ps_sc", bufs=2, space="PSUM") as ps_sc,
        ):
            # ---------------- persistent tiles ----------------
            wq_bf = persist.tile([128, KT, HPC * HD], BF16, tag="wq_bf")
            wk_bf = persist.tile([128, KT, HPC * HD], BF16, tag="wk_bf")
            wv_bf = persist.tile([128, KT, HPC * HD], BF16, tag="wv_bf")
            cos_sb = persist.tile([HD, S], BF16, tag="cos_sb")
            sin_sb = persist.tile([HD, S], BF16, tag="sin_sb")
            mask_bf = persist.tile([128, 128], BF16, tag="mask_bf")
            # q and k share one tile (dim1: 0=q, 1=k) so RoPE's rotate-half
            # swap needs one DMA pair per (span, head) instead of two
            qk_sb = persist.tile([HD, 2, B, HPC, S], BF16, tag="qk_sb")
            # v with an appended ones column (denominator trick)
            v_sb = persist.tile([128, B, TT, HPC, HD + 1], BF16, tag="v_sb")

            # -------- PE warm-up: garbage matmuls while DMA ramps --------
            # HAM un-throttles (1.2 -> 2.4 GHz) only after ~3.4us of
            # sustained PE activity; burn that in during the initial DMA.
            # matmuls on uninitialized SBUF garbage: results land in a PSUM
            # bank that is never read (the first real accumulation into the
            # reused bank has start=True, which overwrites).  No producer
            # dependency, so these issue the moment the engine comes up.
            wtile = warm.tile([128, 644], BF16, tag="wtile")
            # one-column write allocates the tile; the matmuls read a
            # disjoint (uninitialized) region so they have no producer dep
            nc.vector.memset(wtile[:, 0:1], 0.0)
            ps_w = ps_qk.tile([128, SPAN], F32, tag="qk")
            for i in range(20):
                nc.tensor.matmul(
                    ps_w[:], wtile[:, 4:132], wtile[:, 132:644],
                    start=True, stop=True,
                )

            def load_weight(wdram, wbf, pieces=4):
                kstep = KT // pieces
                for p in range(pieces):
                    nc.sync.dma_start(
                        wbf[:, p * kstep:(p + 1) * kstep, :],
                        wdram[:, p * kstep:(p + 1) * kstep, :],
                    )

            def load_xspan(b, span, pieces=1):
                xspan = xpool.tile([128, KT, SPAN], BF16, tag="x_bf")
                kstep = KT // pieces
                sl = slice(span * SPAN, (span + 1) * SPAN)
                for p in range(pieces):
                    ks = slice(p * kstep, (p + 1) * kstep)
                    nc.sync.dma_start(xspan[:, ks, :], xP[b, :, ks, sl])
                return xspan

            def qk_span(b, span, xspan):
                sl = slice(span * SPAN, (span + 1) * SPAN)
                for h in range(HPC):
                    hsl = slice(h * HD, (h + 1) * HD)
                    pf2 = rope.tile([128, 2, SPAN], BF16, tag="pf")
                    for qk, wbf in ((0, wq_bf), (1, wk_bf)):
                        ps = ps_qk.tile([128, SPAN], F32, tag="qk")
                        for kt in range(KT):
                            nc.tensor.matmul(
                                ps[:],
                                wbf[:, kt, hsl],
                                xspan[:, kt, :],
                                start=(kt == 0),
                                stop=(kt == KT - 1),
                            )
                        nc.scalar.copy(pf2[:, qk, :], ps[:])
                    # RoPE on q and k together: out = p*cos + rot(p)*sin
                    # (bf16 math: ~0.2% extra error, 2x DVE throughput)
                    rot2 = rope.tile([128, 2, SPAN], BF16, tag="rot")
                    nc.sync.dma_start(rot2[0:64], pf2[64:128])
                    nc.sync.dma_start(rot2[64:128], pf2[0:64])
                    cos_b = cos_sb[:, sl].unsqueeze(1).broadcast_to(
                        [HD, 2, SPAN])
                    sin_b = sin_sb[:, sl].unsqueeze(1).broadcast_to(
                        [HD, 2, SPAN])
                    nc.vector.tensor_mul(pf2[:], pf2[:], cos_b)
                    nc.vector.tensor_mul(rot2[:], rot2[:], sin_b)
                    nc.vector.tensor_add(
                        qk_sb[:, :, b, h, sl], pf2[:], rot2[:])

            def v_span(b, span, xspan):
                # V projection (natural [token, head*hd] layout).  Emitted
                # AFTER the span's score matmuls: those only need Q/K, and
                # the V matmuls give ScalarE time to chew through the exps.
                for j in range(4):
                    tt = span * 4 + j
                    psv = ps_qk.tile([128, HPC * HD], F32, tag="qk")
                    for kt in range(KT):
                        nc.tensor.matmul(
                            psv[:],
                            xspan[:, kt, j * 128:(j + 1) * 128],
                            wv_bf[:, kt, :],
                            start=(kt == 0),
                            stop=(kt == KT - 1),
                        )
                    nc.vector.tensor_copy(
                        v_sb[:, b, tt, :, 0:HD],
                        psv[:].rearrange("p (h d) -> p h d", h=HPC),
                    )

            def attn_scores(b, h, s):
                q0 = s * SPAN
                nkt = 4 * s + 4  # causal: k tiles 0 .. 4s+3 (always even)
                es = []
                for kp in range(nkt // 2):
                    # two k-tiles share a 2-bank PSUM: one ACTIVATE covers both
                    # exps, amortizing ScalarE's per-op overhead.  Diagonal
                    # k-tiles (jd>0) only compute the causal q-suffix; the
                    # skipped region of the PSUM/e2 tile is never read.
                    psc = ps_sc.tile([128, 2 * SPAN], F32, tag="sc")
                    for half in range(2):
                        kt = 2 * kp + half
                        c0 = max(0, (kt - 4 * s)) * 128  # causal q-col start
                        nc.tensor.matmul(
                            psc[:, half * SPAN + c0:(half + 1) * SPAN],
                            qk_sb[:, 1, b, h, kt * 128:(kt + 1) * 128],
                            qk_sb[:, 0, b, h, q0 + c0:q0 + SPAN],
                            start=True,
                            stop=True,
                        )
                    e2 = epool.tile([128, 2 * SPAN], BF16, tag="e")
                    ec0 = max(0, (2 * kp - 4 * s)) * 128
                    nc.scalar.activation(
                        e2[:, ec0:], psc[:, ec0:],
                        mybir.ActivationFunctionType.Exp, scale=SCALE
                    )
                    for half in range(2):
                        kt = 2 * kp + half
                        jd = kt - 4 * s
                        base = half * SPAN
                        if jd >= 0:  # diagonal block: zero out k > q
                            nc.vector.tensor_mul(
                                e2[:, base + jd * 128:base + (jd + 1) * 128],
                                e2[:, base + jd * 128:base + (jd + 1) * 128],
                                mask_bf[:],
                            )
                        es.append(e2[:, base:base + SPAN])
                return es

            def attn_vmm(b, h, s, es, o_sb):
                for j in range(4):
                    last_kt = 4 * s + j
                    pso = ps_qk.tile([128, SPAN], F32, tag="qk")
                    for kt in range(last_kt + 1):
                        nc.tensor.matmul(
                            pso[:, 0:HD + 1],
                            es[kt][:, j * 128:(j + 1) * 128],
                            v_sb[:, b, kt, h, :],
                            start=(kt == 0),
                            stop=(kt == last_kt),
                        )
                    recip = rpool.tile([128, 1], F32, tag="recip")
                    nc.vector.reciprocal(recip[:], pso[:, HD:HD + 1])
                    nc.vector.tensor_scalar_mul(
                        o_sb[:, h, j, :], pso[:, 0:HD], recip[:]
                    )

            # ---------------- emission schedule ----------------
            # Spans are processed in global order (batch 0 then batch 1);
            # each span's attention chunks run right after its QKV so early
            # x-span DMA waits are filled with attention matmuls instead of
            # idling the PE (which would also re-throttle the HAM clock
            # gate).  Scores stay one chunk ahead of attn@V so the ScalarE
            # exps hide behind TensorE work.
            spans = [(b, s) for b in range(B) for s in range(NSPANS)]
            x_first = xpool.tile([128, KT, SPAN], BF16, tag="x_bf")
            xtiles = {spans[0]: x_first}
            # each dma_start costs ~0.65us of serialized posting on the Sync
            # queue, so early loads are few and coarse: the posting rate, not
            # HBM bandwidth, set the old warm-up critical path
            nc.sync.dma_start(wq_bf[:, 0:8, :], wqP[:, 0:8, :])
            nc.sync.dma_start(x_first[:], xP[0, :, :, 0:SPAN])
            nc.sync.dma_start(wq_bf[:, 8:16, :], wqP[:, 8:16, :])
            nc.sync.dma_start(wk_bf[:, 0:8, :], wkP[:, 0:8, :])
            nc.sync.dma_start(wk_bf[:, 8:16, :], wkP[:, 8:16, :])
            nc.sync.dma_start(cos_sb[:], cosT[:])
            nc.sync.dma_start(sin_sb[:], sinT[:])
            nc.sync.dma_start(mask_bf[:], maskd[:])
            load_weight(wvP, wv_bf, pieces=1)
            xtiles[spans[1]] = load_xspan(*spans[1])
            nc.vector.memset(v_sb[:, :, :, :, HD], 1.0)

            chunks = [(b, h, s) for (b, s) in spans for h in range(HPC)]
            es_map = {}
            o_tiles = {}
            scored = 0  # chunks whose scores have been emitted
            done = 0    # chunks whose attn@V has been emitted

            def emit_vmm(c):
                cb, ch, cs = chunks[c]
                key = (cb, cs)
                if key not in o_tiles:
                    o_tiles[key] = opool.tile([128, HPC, 4, 128], BF16,
                                              tag="o", name=f"o_{cb}_{cs}")
                attn_vmm(cb, ch, cs, es_map.pop(c), o_tiles[key])
                if ch == HPC - 1:  # both chunks of the span done: one DMA
                    nc.sync.dma_start(attnout[cb, cs], o_tiles.pop(key)[:])
            for sp, (b, s) in enumerate(spans):
                xspan = xtiles.pop((b, s))
                qk_span(b, s, xspan)
                if sp < len(spans) - 1:
                    v_span(b, s, xspan)
                    if sp + 2 < len(spans):
                        xtiles[spans[sp + 2]] = load_xspan(*spans[sp + 2])
                    for _ in range(HPC):
                        es_map[scored] = attn_scores(*chunks[scored])
                        scored += 1
                        if scored - done >= 4:
                            emit_vmm(done)
                            done += 1
                else:
                    # last span: score its chunks BEFORE the V projection so
                    # the final (ScalarE-bound) exp stream hides behind the V
                    # matmuls instead of stalling the attn@V drain
                    for _ in range(HPC):
                        es_map[scored] = attn_scores(*chunks[scored])
                        scored += 1
                        if scored - done >= 3:
                            emit_vmm(done)
                            done += 1
                    v_span(b, s, xspan)
            while done < len(chunks):
                emit_vmm(done)
                done += 1
    return nc


# ---------------------------------------------------------------------------
# NEFF 2: output projection, token-parallel
# ---------------------------------------------------------------------------

def build_oproj_nc():
    """out[tok, hout] = attnT.T @ WoT on a 2x4 (token-half x hout-quarter)
    core grid: per-core inputs are 8 MiB of bf16 attention state and 2 MiB of
    bf16 Wo columns, streamed K-tile by K-tile so the loads hide under
    matmuls."""
    nc = bass.Bass(target_bir_lowering=False, debug=False)

    TOKS = (B * S) // 2   # 2048 tokens per core (token half)
    HOUT = HIDDEN // 4    # 512 output channels per core (hout quarter)
    aP = nc.dram_tensor("aP", [128, KT, TOKS], BF16, kind="ExternalInput")
    woP = nc.dram_tensor("woP", [128, KT, HOUT], BF16, kind="ExternalInput")
    out = nc.dram_tensor("out", [TOKS, HOUT], F32, kind="ExternalOutput")

    with tile.TileContext(nc) as tc:
        with (
            tc.tile_pool(name="warm", bufs=1) as warm,
            tc.tile_pool(name="persist", bufs=1) as persist,
            tc.tile_pool(name="outp", bufs=3) as outp,
            tc.tile_pool(name="psum", bufs=8, space="PSUM") as psum,
        ):
            a_bf = persist.tile([128, KT, TOKS], BF16, tag="a_bf")
            wo_bf = persist.tile([128, KT, HOUT], BF16, tag="wo_bf")

            # PE warm-up while input DMA ramps
            wtile = warm.tile([128, 644], BF16, tag="wtile")
            nc.vector.memset(wtile[:, 0:1], 0.0)
            ps_w = psum.tile([128, HOUT], F32, tag="ps", name="ps_warm")
            for i in range(16):
                nc.tensor.matmul(
                    ps_w[:], wtile[:, 4:132], wtile[:, 132:644],
                    start=True, stop=True,
                )

            def flush(m_tiles, fuse=False):
                ms = list(m_tiles)
                if fuse:  # one copy+DMA pair for the whole group (tail trim)
                    o2 = outp.tile([128, len(ms), HOUT], F32, tag="o2",
                                   name=f"o2_{ms[0]}")
                    for k, m in enumerate(ms):
                        nc.vector.tensor_copy(o2[:, k, :], ps_grid[m][:])
                    dst = out[ms[0] * 128:(ms[0] + len(ms)) * 128, :]
                    nc.sync.dma_start(
                        dst.rearrange("(a p) h -> p a h", p=128), o2[:]
                    )
                    return
                for m in ms:
                    o = outp.tile([128, HOUT], F32, tag="o", name=f"o_{m}")
                    nc.vector.tensor_copy(o[:], ps_grid[m][:])
                    nc.sync.dma_start(out[m * 128:(m + 1) * 128, :], o[:])

            # pass 1: token tiles 0-7, streaming Wo in per K tile (JIT)
            ps_grid = {m: psum.tile([128, HOUT], F32, tag="ps", name=f"ps_{m}")
                       for m in range(8)}
            nc.sync.dma_start(a_bf[:, 0, :], aP[:, 0, :])
            nc.sync.dma_start(wo_bf[:, 0, :], woP[:, 0, :])
            nc.sync.dma_start(a_bf[:, 1, :], aP[:, 1, :])
            for kt in range(KT):
                if kt + 1 < KT:
                    nc.sync.dma_start(wo_bf[:, kt + 1, :], woP[:, kt + 1, :])
                if kt + 2 < KT:
                    nc.sync.dma_start(a_bf[:, kt + 2, :], aP[:, kt + 2, :])
                for m in range(8):
                    nc.tensor.matmul(
                        ps_grid[m][:],
                        a_bf[:, kt, m * 128:(m + 1) * 128],
                        wo_bf[:, kt, :],
                        start=(kt == 0),
                        stop=(kt == KT - 1),
                    )
            flush(range(8))
            # passes 2a/2b: token tiles 8-11 then 12-15 from the cached bf16
            # weights; splitting lets the first flush DMA overlap the second
            # sub-pass instead of sitting in the kernel tail
            for lo, n in ((8, 4), (12, 2), (14, 2)):
                ps_grid = {m: psum.tile([128, HOUT], F32, tag="ps",
                                        name=f"ps_{m}")
                           for m in range(lo, lo + n)}
                for kt in range(KT):
                    for m in range(lo, lo + n):
                        nc.tensor.matmul(
                            ps_grid[m][:],
                            a_bf[:, kt, m * 128:(m + 1) * 128],
                            wo_bf[:, kt, :],
                            start=(kt == 0),
                            stop=(kt == KT - 1),
                        )
                flush(range(lo, lo + n), fuse=(lo == 14))
    return nc


# ---------------------------------------------------------------------------
# host driver
# ---------------------------------------------------------------------------

_NC_CACHE = {}


def _get_ncs():
    if "attn" not in _NC_CACHE:
        _NC_CACHE["attn"] = build_attn_nc()
        _NC_CACHE["oproj"] = build_oproj_nc()
    return _NC_CACHE["attn"], _NC_CACHE["oproj"]


def _rope_tables():
    inv_freq = 1.0 / (10000.0 ** (np.arange(0, HD, 2, dtype=np.float32) / HD))
    t = np.arange(S, dtype=np.float32)
    freqs = np.einsum("i,j->ij", t, inv_freq)  # [S, HD/2]
    emb = np.concatenate([freqs, freqs], axis=-1)  # [S, HD]
    cos = np.cos(emb).astype(np.float32)
    sin = np.sin(emb).astype(np.float32)
    cosT = np.ascontiguousarray(cos.T)  # [HD, S]
    sinT = np.ascontiguousarray(sin.T)
    sinT_signed = sinT.copy()
    sinT_signed[0:64, :] *= -1.0  # fold rotate_half's negation into the table
    return cosT.astype(NP_BF16), sinT_signed.astype(NP_BF16)


def _w_partition_major(Wslice):
    """[256 outs, HIDDEN] f32 -> [128, KT, 256] bf16, partition-major."""
    # w[h, ko*128 + p] -> out[p, ko, h]
    return np.ascontiguousarray(
        Wslice.T.reshape(KT, 128, HPC * HD).transpose(1, 0, 2)
    ).astype(NP_BF16)


def kernel(x, Wq, Wk, Wv, Wo):
    x = np.asarray(x, dtype=np.float32)
    Wq = np.asarray(Wq, dtype=np.float32)
    Wk = np.asarray(Wk, dtype=np.float32)
    Wv = np.asarray(Wv, dtype=np.float32)
    Wo = np.asarray(Wo, dtype=np.float32)

    nc1, nc2 = _get_ncs()
    core_ids = list(range(N_CORES))
    trace = bool(os.environ.get("BASS_TRACE"))

    cosT, sinT_signed = _rope_tables()
    mask = np.triu(np.ones((128, 128), dtype=np.float32)).astype(
        NP_BF16
    )  # mask[k,q]=1 iff k<=q
    # x[b, t, ko*128+p] -> xP[b, p, ko, t]  (partition-major, bf16)
    xP = np.ascontiguousarray(
        x.reshape(B, S, KT, 128).transpose(0, 3, 2, 1)
    ).astype(NP_BF16)

    in_maps1 = []
    for c in range(N_CORES):
        csl = slice(c * HPC * HD, (c + 1) * HPC * HD)
        in_maps1.append(
            {
                "xP": xP,
                "wqP": _w_partition_major(Wq[csl, :]),
                "wkP": _w_partition_major(Wk[csl, :]),
                "wvP": _w_partition_major(Wv[csl, :]),
                "cosT": cosT,
                "sinT": sinT_signed,
                "mask": mask,
            }
        )

    LAST_RESULTS.clear()
    res1 = run_bass_kernel_spmd(nc1, in_maps1, core_ids=core_ids, trace=trace)
    LAST_RESULTS.append(res1)

    # host relayout: per-head attention outputs -> attnT [HIDDEN, B*S]
    arr = np.stack([res1.results[c]["attnout"] for c in range(N_CORES)])
    # axes: (core, b, s, ql, h, qt, dl) -> d = core*256 + h*128 + dl,
    #       tok = b*2048 + s*512 + qt*128 + ql
    attnT = np.ascontiguousarray(
        arr.transpose(0, 4, 6, 1, 2, 5, 3).reshape(HIDDEN, B * S)
    )
    # attnT[ko*128+p, tok] -> aP[p, ko, tok] per token half
    aP_full = np.ascontiguousarray(
        attnT.reshape(KT, 128, B * S).transpose(1, 0, 2)
    )
    # Wo[ho, ko*128+p] -> woP[p, ko, ho] per hout quarter
    woP_full = np.ascontiguousarray(
        Wo.T.reshape(KT, 128, HIDDEN).transpose(1, 0, 2)
    ).astype(NP_BF16)

    TOKS = (B * S) // 2
    HOUT = HIDDEN // 4
    in_maps2 = []
    for c in range(N_CORES):
        ti, hj = c // 4, c % 4
        in_maps2.append(
            {
                "aP": np.ascontiguousarray(
                    aP_full[:, :, ti * TOKS:(ti + 1) * TOKS]
                ),
                "woP": np.ascontiguousarray(
                    woP_full[:, :, hj * HOUT:(hj + 1) * HOUT]
                ),
            }
        )
    res2 = run_bass_kernel_spmd(nc2, in_maps2, core_ids=core_ids, trace=trace)
    LAST_RESULTS.append(res2)

    out = np.empty((B * S, HIDDEN), dtype=np.float32)
    for c in range(N_CORES):
        ti, hj = c // 4, c % 4
        out[ti * TOKS:(ti + 1) * TOKS, hj * HOUT:(hj + 1) * HOUT] = (
            res2.results[c]["out"]
        )
    return np.ascontiguousarray(out.reshape(B, S, HIDDEN), dtype=np.float32)
